# revision 1
# baseline (speedup 1.0000x reference)
"""Trainium2 Bass kernel for conv-qkv rank-1 attention.

out = gamma * q * sum(k*v) + x, where q,k,v are per-time-slice 3x3 convs
(C=64 -> C=64) of x [B=8, C=64, T=16, W=64, H=64].

Sharding: data-parallel over B across 8 cores (1 example/core), conv
weights replicated. No cross-core communication.

Per-core schedule: T slices processed in pairs; slice t lives on SBUF
partitions 0-63, slice t+1 on partitions 64-127, giving two concurrent
PE row-group chains (K=64 each). Each 3x3 conv = 9 shifted matmul taps
(+1 bias tap with an all-ones rhs) accumulated in PSUM. Stationary
[Wq|Wk] (M=128) produces q,k in one bank; Wv (M=64) is column-placed so
k and v land on the same partitions for the fused DVE k*v+reduce.
Matmuls run in float32r (FP22 truncation on read, 1 cycle/row).
"""

import numpy as np

import concourse.bacc as bacc
import concourse.bass as bass
import concourse.mybir as mybir
import concourse.tile as tile
from concourse import bass_utils

F32 = mybir.dt.float32
F32R = mybir.dt.float32r
ALU = mybir.AluOpType

B, C, T, W, H = 8, 64, 16, 64, 64
WP, HP = W + 2, H + 2          # padded slice dims
import os
NPAIR = int(os.environ.get("BASS_NPAIR", T // 2))  # slice pairs per core
RB = 8                         # W-rows per pixel block
NBLK = W // RB                 # pixel blocks per slice
BN = RB * H                    # moving free dim per matmul (512)
NTAP = 10                      # 9 conv taps + 1 bias tap


def _round22(a: np.ndarray) -> np.ndarray:
    """Round fp32 to 11 mantissa bits so the PE's FP22 read-truncation is
    exact (unbiased quantization instead of truncation)."""
    u = np.ascontiguousarray(a, np.float32).view(np.uint32).astype(np.uint64)
    u = ((u + 0x800) & 0xFFFFF000).astype(np.uint32)
    return u.view(np.float32)


def _pack_weights(wq, wk, wv, bq, bk, bv):
    """Pack stationary operands.

    wqk [128, 10, 128]: partitions 0-63 = chain-low taps ([Wq | Wk] so q
    lands on psum partitions 0-63, matching x_t's partitions), partitions
    64-127 = chain-high taps ([Wk | Wq], q on partitions 64-127). Tap 9 is
    the bias tap (row 0 = biases, used with an all-ones rhs).
    wv2 [128, 10, 64]: Wv taps for both chains (same values).
    """
    def taps(w):  # [O, I, 1, 3, 3] -> [I, 9, O]
        return np.ascontiguousarray(
            w.reshape(C, C, 9).transpose(1, 2, 0), np.float32)

    wq_t, wk_t, wv_t = taps(wq), taps(wk), taps(wv)
    # [Wk | Wq] for both chains: k lands on psum partitions 0-63 (the
    # custom DVE reduce op requires base partition 0), q on 64-127
    wqk = np.zeros((128, NTAP, 128), np.float32)
    wqk[0:64, 0:9, 0:64] = wk_t
    wqk[0:64, 0:9, 64:128] = wq_t
    wqk[64:128, 0:9, 0:64] = wk_t
    wqk[64:128, 0:9, 64:128] = wq_t
    wqk[0, 9, 0:64] = bk
    wqk[0, 9, 64:128] = bq
    wqk[64, 9, 0:64] = bk
    wqk[64, 9, 64:128] = bq

    # v stationary is [Wv | Wv] (M=128): the duplicated column half costs
    # nothing (M=64 would leave the array half idle) and lets every matmul
    # use column position 0, which fp32r codegen requires
    wv2 = np.zeros((128, NTAP, 128), np.float32)
    wv2[0:64, 0:9, 0:64] = wv_t
    wv2[0:64, 0:9, 64:128] = wv_t
    wv2[64:128, 0:9, 0:64] = wv_t
    wv2[64:128, 0:9, 64:128] = wv_t
    wv2[0, 9, 0:64] = bv
    wv2[0, 9, 64:128] = bv
    wv2[64, 9, 0:64] = bv
    wv2[64, 9, 64:128] = bv
    return _round22(wqk), _round22(wv2)


def _emit(nc, tc, x_d, wqk_d, wv_d, gam_d, ones_d, zer_d, out_d, ctx):
    const = ctx.enter_context(tc.tile_pool(name="const", bufs=1))
    state = ctx.enter_context(tc.tile_pool(name="state", bufs=1))
    psum = ctx.enter_context(
        tc.tile_pool(name="psum", bufs=2, space=bass.MemorySpace.PSUM))
    vpool = ctx.enter_context(tc.tile_pool(name="vpool", bufs=2))

    wqk_t = const.tile([128, NTAP, 128], F32R, tag="wqk")
    wv_t = const.tile([128, NTAP, 128], F32R, tag="wv")
    gam_t = const.tile([128, 1], F32, tag="gam")
    ones_t = const.tile([128, BN], F32R, tag="ones")

    nc.sync.dma_start(wqk_t[:], wqk_d[:])
    nc.sync.dma_start(wv_t[:], wv_d[:])
    nc.sync.dma_start(gam_t[:], gam_d[:])
    nc.sync.dma_start(ones_t[:], ones_d[:])

    xp = [state.tile([128, WP, HP], F32R, tag=f"xp{i}", name=f"xp{i}") for i in range(3)]
    qs = [state.tile([128, W * H], F32, tag=f"qs{i}", name=f"qs{i}") for i in range(2)]
    ot = [state.tile([128, W * H], F32, tag=f"ot{i}", name=f"ot{i}") for i in range(2)]
    scr = state.tile([128, BN], F32, tag="scr")
    sparts = [state.tile([64, 2, NBLK], F32, tag=f"sp{i}", name=f"sp{i}") for i in range(2)]
    sgam = [state.tile([64, 2], F32, tag=f"sg{i}", name=f"sg{i}") for i in range(2)]
    sfin = [state.tile([128, 1], F32, tag=f"sf{i}", name=f"sf{i}") for i in range(2)]

    # zero the padding ring of both x buffers once (gpsimd memset does not
    # take f32r, so DMA from a host-provided zero vector); interior DMAs
    # never touch the ring
    for t_ in xp:
        nc.sync.dma_start(t_[:, 0, :], zer_d[:, :])
        nc.sync.dma_start(t_[:, WP - 1, :], zer_d[:, :])
        nc.sync.dma_start(t_[:, :, 0], zer_d[:, 0:WP])
        nc.sync.dma_start(t_[:, :, HP - 1], zer_d[:, 0:WP])

    def load_pair(p):
        t_ = xp[p % 3]
        nc.sync.dma_start(t_[0:64, 1:1 + W, 1:1 + H], x_d[:, 2 * p])
        nc.sync.dma_start(t_[64:128, 1:1 + W, 1:1 + H], x_d[:, 2 * p + 1])

    load_pair(0)
    if NPAIR > 1:
        load_pair(1)

    for p in range(NPAIR):
        pb = p % 2
        xp_, qs_, ot_ = xp[p % 3], qs[pb], ot[pb]

        # prefetch two pairs ahead; emitted before this pair's s-swap DMA
        # so the serial sync queue never holds the x-load behind it
        if p + 2 < NPAIR:
            load_pair(p + 2)

        for j in range(NBLK):
            qk_lo = psum.tile([128, BN], F32, tag="qk_lo")
            qk_hi = psum.tile([128, BN], F32, tag="qk_hi")
            v_lo = psum.tile([128, BN], F32, tag="v_lo", name="v_lo")
            v_hi = psum.tile([128, BN], F32, tag="v_hi", name="v_hi")

            def rhs(half, tap):
                if tap == 9:
                    return ones_t[64 * half:64 * half + 64, :]
                dy, dx = tap // 3, tap % 3
                r0 = j * RB + dy
                return xp_[64 * half:64 * half + 64,
                           r0:r0 + RB, dx:dx + H]

            for tap in range(NTAP):
                st, sp = tap == 0, tap == NTAP - 1
                nc.tensor.matmul(
                    qk_lo[:, :],
                    wqk_t[0:64, tap, :],
                    rhs(0, tap), start=st, stop=sp)
                nc.tensor.matmul(
                    qk_hi[:, :],
                    wqk_t[64:128, tap, :],
                    rhs(1, tap), start=st, stop=sp)
            for tap in range(NTAP):
                st, sp = tap == 0, tap == NTAP - 1
                nc.tensor.matmul(
                    v_lo[:, :],
                    wv_t[0:64, tap, :],
                    rhs(0, tap), start=st, stop=sp)
                nc.tensor.matmul(
                    v_hi[:, :],
                    wv_t[64:128, tap, :],
                    rhs(1, tap), start=st, stop=sp)

            # evacuate q and v on ScalarE (DVE may read only one PSUM
            # operand, so v must reach SBUF before the fused k*v reduce).
            # q_t moves partitions 64-127 -> 0-63 to line up with x_t.
            if os.environ.get("BASS_QCROSS", "1") == "1":
                nc.scalar.copy(qs_[0:64, j * BN:(j + 1) * BN], qk_lo[64:128, :])
            else:
                nc.scalar.copy(qs_[0:64, j * BN:(j + 1) * BN], qk_lo[0:64, :])
            nc.scalar.copy(qs_[64:128, j * BN:(j + 1) * BN], qk_hi[64:128, :])
            vsb_lo = vpool.tile([64, BN], F32, tag="vsb_lo", name="vsb_lo")
            vsb_hi = vpool.tile([64, BN], F32, tag="vsb_hi", name="vsb_hi")
            nc.scalar.copy(vsb_lo[:, :], v_lo[0:64, :])
            nc.scalar.copy(vsb_hi[:, :], v_hi[0:64, :])

            # fused k*v multiply + pixel-sum (k from PSUM at base partition
            # 0 -- the custom DVE op requires it; v from SBUF)
            if os.environ.get("BASS_TTR", "1") == "1":
                # native TensorScalarPtr with accumulate: one DVE pass does
                # k*v and the pixel-sum
                nc.vector.scalar_tensor_tensor(
                    out=scr[0:64, :], in0=qk_lo[0:64, :], scalar=1.0,
                    in1=vsb_lo[:, :], op0=ALU.mult, op1=ALU.mult,
                    accum_out=sparts[pb][:, 0, j:j + 1])
                nc.vector.scalar_tensor_tensor(
                    out=scr[0:64, :], in0=qk_hi[0:64, :], scalar=1.0,
                    in1=vsb_hi[:, :], op0=ALU.mult, op1=ALU.mult,
                    accum_out=sparts[pb][:, 1, j:j + 1])
            else:
                nc.vector.tensor_tensor(
                    out=scr[0:64, :], in0=qk_lo[0:64, :], in1=vsb_lo[:, :],
                    op=ALU.mult)
                nc.vector.reduce_sum(sparts[pb][:, 0, j:j + 1], scr[0:64, :],
                                     axis=mybir.AxisListType.X)
                nc.vector.tensor_tensor(
                    out=scr[0:64, :], in0=qk_hi[0:64, :], in1=vsb_hi[:, :],
                    op=ALU.mult)
                nc.vector.reduce_sum(sparts[pb][:, 1, j:j + 1], scr[0:64, :],
                                     axis=mybir.AxisListType.X)

        nc.vector.reduce_sum(sgam[pb][:, :], sparts[pb][:, :, :],
                             axis=mybir.AxisListType.X)
        nc.vector.tensor_scalar_mul(sgam[pb][:, :], sgam[pb][:, :],
                                    gam_t[0:64, 0:1])
        # s_{t+1} is accumulated on partitions 0-63 but q_{t+1}/x_{t+1}
        # live on 64-127: move it with a tiny sbuf->sbuf DMA
        if os.environ.get("BASS_SWAPDMA", "1") == "1":
            nc.sync.dma_start(sfin[pb][64:128, :], sgam[pb][:, 1:2])
        else:
            nc.vector.tensor_copy(sfin[pb][0:64, :], sgam[pb][:, 1:2])

        for j in range(NBLK):
            # out = (q * (gamma*s)) + x, fused
            nc.vector.scalar_tensor_tensor(
                out=ot_[0:64, j * BN:(j + 1) * BN],
                in0=qs_[0:64, j * BN:(j + 1) * BN],
                scalar=sgam[pb][:, 0:1],
                in1=xp_[0:64, 1 + j * RB:1 + (j + 1) * RB, 1:1 + H].bitcast(F32),
                op0=ALU.mult, op1=ALU.add)
            nc.vector.scalar_tensor_tensor(
                out=ot_[64:128, j * BN:(j + 1) * BN],
                in0=qs_[64:128, j * BN:(j + 1) * BN],
                scalar=sfin[pb][64:128, 0:1],
                in1=xp_[64:128, 1 + j * RB:1 + (j + 1) * RB, 1:1 + H].bitcast(F32),
                op0=ALU.mult, op1=ALU.add)

        nc.gpsimd.dma_start(out_d[:, 2 * p], ot_[0:64, :])
        nc.gpsimd.dma_start(out_d[:, 2 * p + 1], ot_[64:128, :])


_ONES = np.ones((128, BN), np.float32)
_ZER = np.zeros((128, HP), np.float32)

_CACHE = {}


def _build():
    if "nc" in _CACHE:
        return _CACHE["nc"]
    nc = bacc.Bacc("TRN2", target_bir_lowering=False, debug=False,
                   enable_asserts=False, num_devices=8)
    x_d = nc.dram_tensor("x", (C, T, W, H), F32R, kind="ExternalInput").ap()
    wqk_d = nc.dram_tensor("wqk", (128, NTAP, 128), F32R,
                           kind="ExternalInput").ap()
    wv_d = nc.dram_tensor("wv2", (128, NTAP, 128), F32R,
                          kind="ExternalInput").ap()
    gam_d = nc.dram_tensor("gamma_bc", (128, 1), F32,
                           kind="ExternalInput").ap()
    ones_d = nc.dram_tensor("ones", (128, BN), F32R,
                            kind="ExternalInput").ap()
    zer_d = nc.dram_tensor("zer", (128, HP), F32R,
                           kind="ExternalInput").ap()
    out_d = nc.dram_tensor("out", (C, T, W, H), F32,
                           kind="ExternalOutput").ap()
    from contextlib import ExitStack
    with tile.TileContext(nc) as tc, ExitStack() as ctx:
        _emit(nc, tc, x_d, wqk_d, wv_d, gam_d, ones_d, zer_d, out_d, ctx)
    nc.compile()
    _CACHE["nc"] = nc
    return nc


def run_spmd(x, wq, wk, wv, bq, bk, bv, gamma, trace=False, **kw):
    nc = _build()
    wqk, wv2 = _pack_weights(
        np.asarray(wq, np.float32), np.asarray(wk, np.float32),
        np.asarray(wv, np.float32), np.asarray(bq, np.float32),
        np.asarray(bk, np.float32), np.asarray(bv, np.float32))
    gam = np.full((128, 1), np.float32(np.asarray(gamma).reshape(-1)[0]),
                  np.float32)
    x = np.asarray(x, np.float32)
    in_maps = [
        {"x": np.ascontiguousarray(x[b]), "wqk": wqk, "wv2": wv2,
         "gamma_bc": gam, "ones": _ONES, "zer": _ZER}
        for b in range(B)
    ]
    res = bass_utils.run_bass_kernel_spmd(
        nc, in_maps, core_ids=list(range(B)), trace=trace, **kw)
    out = np.stack([res.results[b]["out"] for b in range(B)], axis=0)
    return out, res


def kernel(x, wq, wk, wv, bq, bk, bv, gamma):
    out, _ = run_spmd(x, wq, wk, wv, bq, bk, bv, gamma)
    return out



# revision 8
# speedup vs baseline: 1.1190x; 1.1190x over previous
"""Trainium2 Bass kernel for conv-qkv rank-1 attention.

out = gamma * q * sum((k+bk)*(v+bv)) + x, where q,k,v are per-time-slice
3x3 convs (C=64 -> C=64) of x [B=8, C=64, T=16, W=64, H=64].

Sharding: data-parallel over B across 8 cores (1 example/core), conv
weights replicated. No cross-core communication.

Per-core schedule (v2, tap-paired bf16):
Each slice keeps TWO copies of x in one SBUF tile [128, 66, 66]:
even slices [shift | plain], odd slices [plain | shift], where "shift"
is x offset one column so that a single 128-partition moving AP delivers
two different conv taps on the two partition halves. A 3x3 conv then
costs 3 K=128 "pair" matmuls (taps (dy,0)+(dy,1)) plus 3 K=64 "single"
matmuls (taps (dy,2)); singles of even/odd slices sit on disjoint PE
row groups and run concurrently (2-way row tiling), v-chain matmuls of
even/odd slices sit on disjoint column groups (2-way col tiling).
Per block of 512 pixels and slice pair this is 15 PE slots of N=512
vs 20 in the direct scheme.

Biases never enter the PE: bq/bv are folded into the PSUM->SBUF
evacuation on ScalarE (activation bias), and bk's contribution
bk*sum(v+bv) is recovered from the v-evacuation's accum_out.
The final out = q*(gamma*s) + x runs on GpSimd (Pool) so the DVE
queue never stalls the next pair's PSUM turnaround.

All matmul operands are bf16 (hosts casts x with round-to-nearest);
PSUM accumulation stays fp32.
"""

import os

import numpy as np
import ml_dtypes

import concourse.bacc as bacc
import concourse.bass as bass
import concourse.mybir as mybir
import concourse.tile as tile
from concourse import bass_utils

F32 = mybir.dt.float32
BF16 = mybir.dt.bfloat16
ALU = mybir.AluOpType
ACT = mybir.ActivationFunctionType

B, C, T, W, H = 8, 64, 16, 64, 64
WP, HP = W + 2, H + 2          # padded slice dims
NPAIR = int(os.environ.get("BASS_NPAIR", T // 2))  # slice pairs per core
RB = 8                         # W-rows per pixel block
NBLK = W // RB                 # pixel blocks per slice
BN = RB * H                    # moving free dim per matmul (512)
NABUF = 4                      # A-tile buffers per parity (4-deep rotation)
FINCH = 2                      # final-pass chunks per slice


def _bf16(a):
    return np.asarray(a, np.float32).astype(ml_dtypes.bfloat16)


def _pack_weights(wq, wk, wv):
    """Pack stationary operands (bf16).

    Moving-data convention: a pair matmul reads the full 128-partition AP
    at (r0=j*RB+dy, dx=0): on even slices the low half (shift copy)
    delivers tap (dy,1) and the high half (plain) tap (dy,0); odd slices
    are mirrored. Single matmuls read the shift copy at dx=1 -> tap
    (dy,2): even from partitions 0-63, odd from 64-127.
    kq column layout: even [Wk | Wq] (q lands on PSUM 64:128 = the
    x-plain half), odd [Wq | Wk].
    """
    def taps(w):  # [O, I, 1, 3, 3] -> tap(dy,dx) = [I, O]
        return np.ascontiguousarray(w.reshape(C, C, 3, 3).transpose(1, 2, 3, 0),
                                    np.float32)

    q_t, k_t, v_t = taps(wq), taps(wk), taps(wv)

    kq_pair = np.zeros((2, 3, 128, 128), np.float32)
    v_pair = np.zeros((2, 3, 128, 64), np.float32)
    for dy in range(3):
        # even parity
        kq_pair[0, dy, 0:64, 0:64] = k_t[:, dy, 1]
        kq_pair[0, dy, 0:64, 64:128] = q_t[:, dy, 1]
        kq_pair[0, dy, 64:128, 0:64] = k_t[:, dy, 0]
        kq_pair[0, dy, 64:128, 64:128] = q_t[:, dy, 0]
        v_pair[0, dy, 0:64, :] = v_t[:, dy, 1]
        v_pair[0, dy, 64:128, :] = v_t[:, dy, 0]
        # odd parity
        kq_pair[1, dy, 0:64, 0:64] = q_t[:, dy, 0]
        kq_pair[1, dy, 0:64, 64:128] = k_t[:, dy, 0]
        kq_pair[1, dy, 64:128, 0:64] = q_t[:, dy, 1]
        kq_pair[1, dy, 64:128, 64:128] = k_t[:, dy, 1]
        v_pair[1, dy, 0:64, :] = v_t[:, dy, 0]
        v_pair[1, dy, 64:128, :] = v_t[:, dy, 1]

    kq_sing = np.zeros((3, 128, 128), np.float32)
    v_sing = np.zeros((3, 128, 64), np.float32)
    for dy in range(3):
        kq_sing[dy, 0:64, 0:64] = k_t[:, dy, 2]
        kq_sing[dy, 0:64, 64:128] = q_t[:, dy, 2]
        kq_sing[dy, 64:128, 0:64] = q_t[:, dy, 2]
        kq_sing[dy, 64:128, 64:128] = k_t[:, dy, 2]
        v_sing[dy, 0:64, :] = v_t[:, dy, 2]
        v_sing[dy, 64:128, :] = v_t[:, dy, 2]

    return _bf16(kq_pair), _bf16(kq_sing), _bf16(v_pair), _bf16(v_sing)


def _emit(nc, tc, x_d, wkqp_d, wkqs_d, wvp_d, wvs_d, bias_d, out_d, ctx):
    const = ctx.enter_context(tc.tile_pool(name="const", bufs=1))
    state = ctx.enter_context(tc.tile_pool(name="state", bufs=1))
    psum = ctx.enter_context(
        tc.tile_pool(name="psum", bufs=2, space=bass.MemorySpace.PSUM))
    vpool = ctx.enter_context(tc.tile_pool(name="vpool", bufs=2))

    wkqp_t = const.tile([128, 2, 3, 128], BF16, tag="wkqp")
    wkqs_t = const.tile([128, 3, 128], BF16, tag="wkqs")
    wvp_t = const.tile([128, 2, 3, 64], BF16, tag="wvp")
    wvs_t = const.tile([128, 3, 64], BF16, tag="wvs")
    bias_t = const.tile([128, 4], F32, tag="bias")  # bq, bv, bk*gam, gam

    nc.sync.dma_start(wkqp_t[:], wkqp_d[:])
    nc.sync.dma_start(wkqs_t[:], wkqs_d[:])
    nc.sync.dma_start(wvp_t[:], wvp_d[:])
    nc.sync.dma_start(wvs_t[:], wvs_d[:])
    nc.sync.dma_start(bias_t[:], bias_d[:])

    # A tiles: [shift | plain] for even slices, [plain | shift] for odd.
    ae = [state.tile([128, WP, HP], BF16, tag=f"ae{i}", name=f"ae{i}")
          for i in range(NABUF)]
    ao = [state.tile([128, WP, HP], BF16, tag=f"ao{i}", name=f"ao{i}")
          for i in range(NABUF)]
    qs = [state.tile([128, W * H], BF16, tag=f"qs{i}", name=f"qs{i}")
          for i in range(2)]
    ot = [state.tile([128, W * H], BF16, tag=f"ot{i}", name=f"ot{i}")
          for i in range(2)]
    scr = state.tile([128, BN], F32, tag="scr")
    sparts = [state.tile([128, NBLK], F32, tag=f"sp{i}", name=f"sp{i}")
              for i in range(2)]
    vsum = [state.tile([128, NBLK], F32, tag=f"vs{i}", name=f"vs{i}")
            for i in range(2)]
    t1 = [state.tile([128, 1], F32, tag=f"t1{i}", name=f"t1{i}")
          for i in range(2)]
    vs1 = [state.tile([128, 1], F32, tag=f"vs1{i}", name=f"vs1{i}")
           for i in range(2)]
    sg = [state.tile([128, 1], F32, tag=f"sg{i}", name=f"sg{i}")
          for i in range(2)]
    sf = [state.tile([128, 1], F32, tag=f"sf{i}", name=f"sf{i}")
          for i in range(2)]

    # zero the padding rings once; interior DMAs never touch them.
    # shift halves read cols 0..64 (ring col 64), plain halves cols 0..64
    # (ring col 0); rows 0 and WP-1 are ring for both.
    for tl, shift_lo in [(a, True) for a in ae] + [(a, False) for a in ao]:
        eng = nc.gpsimd if shift_lo else nc.vector
        eng.memset(tl[:, 0, :].bitcast(mybir.dt.uint16), 0)
        eng.memset(tl[:, WP - 1, :].bitcast(mybir.dt.uint16), 0)
        if shift_lo:  # even: shift on 0:64 (ring col W), plain on 64:128
            eng.memset(tl[0:64, :, W].bitcast(mybir.dt.uint16), 0)
            eng.memset(tl[64:128, :, 0].bitcast(mybir.dt.uint16), 0)
        else:         # odd: plain on 0:64, shift on 64:128
            eng.memset(tl[0:64, :, 0].bitcast(mybir.dt.uint16), 0)
            eng.memset(tl[64:128, :, W].bitcast(mybir.dt.uint16), 0)

    def load_pair(p, startup=False):
        te, to = ae[p % NABUF], ao[p % NABUF]
        se, so = 2 * p, 2 * p + 1
        qe = [nc.sync, nc.sync, nc.gpsimd, nc.gpsimd]
        if startup:
            qe = [[nc.sync, nc.sync, nc.gpsimd, nc.gpsimd],
                  [nc.scalar, nc.scalar, nc.sync, nc.gpsimd]][p]
        qe[0].dma_start(te[0:64, 1:1 + W, 0:H], x_d[:, se])       # shift
        qe[1].dma_start(te[64:128, 1:1 + W, 1:1 + H], x_d[:, se])  # plain
        qe[2].dma_start(to[0:64, 1:1 + W, 1:1 + H], x_d[:, so])    # plain
        qe[3].dma_start(to[64:128, 1:1 + W, 0:H], x_d[:, so])      # shift

    load_pair(0, startup=True)
    if NPAIR > 1:
        load_pair(1, startup=True)

    kvo_mode = os.environ.get("BASS_KVO", "stt")
    fin_eng = nc.vector  # Pool lacks TensorScalarPtr; bf16 gives 2x DVE rate

    for p in range(NPAIR):
        pb = p % 2
        ae_, ao_ = ae[p % NABUF], ao[p % NABUF]
        qs_, ot_ = qs[pb], ot[pb]

        if p + 2 < NPAIR:
            load_pair(p + 2)

        for j in range(NBLK):
            kqE = psum.tile([128, BN], F32, tag="kqE", name="kqE")
            kqO = psum.tile([128, BN], F32, tag="kqO", name="kqO")
            V = psum.tile([128, BN], F32, tag="V", name="V")

            def mov(tl, dy, dx, lo=None):
                r0 = j * RB + dy
                if lo is None:
                    return tl[:, r0:r0 + RB, dx:dx + H]
                if lo:
                    return tl[0:64, r0:r0 + RB, dx:dx + H]
                return tl[64:128, r0:r0 + RB, dx:dx + H]

            # kq pair taps: K=128, full array, serial
            for dy in range(3):
                nc.tensor.matmul(kqE[:, :], wkqp_t[:, 0, dy, :],
                                 mov(ae_, dy, 0), start=(dy == 0), stop=False)
                nc.tensor.matmul(kqO[:, :], wkqp_t[:, 1, dy, :],
                                 mov(ao_, dy, 0), start=(dy == 0), stop=False)
            # kq single taps: K=64, even rows 0-63 / odd rows 64-127,
            # 2-way row-tiled concurrent
            for dy in range(3):
                nc.tensor.matmul(kqE[:, :], wkqs_t[0:64, dy, :],
                                 mov(ae_, dy, 1, lo=True),
                                 start=False, stop=(dy == 2))
                nc.tensor.matmul(kqO[:, :], wkqs_t[64:128, dy, :],
                                 mov(ao_, dy, 1, lo=False),
                                 start=False, stop=(dy == 2))
            # v pair taps: K=128 M=64, even cols 0-63 / odd cols 64-127,
            # 2-way col-tiled concurrent
            for dy in range(3):
                nc.tensor.matmul(V[0:64, :], wvp_t[:, 0, dy, :],
                                 mov(ae_, dy, 0), start=(dy == 0), stop=False,
                                 skip_group_check=True)
                nc.tensor.matmul(V[64:128, :], wvp_t[:, 1, dy, :],
                                 mov(ao_, dy, 0), start=(dy == 0), stop=False,
                                 skip_group_check=True)
            # v single taps: K=64 M=64, quadrants (0,0) and (64,64)
            for dy in range(3):
                nc.tensor.matmul(V[0:64, :], wvs_t[0:64, dy, :],
                                 mov(ae_, dy, 1, lo=True),
                                 start=False, stop=(dy == 2),
                                 skip_group_check=True)
                nc.tensor.matmul(V[64:128, :], wvs_t[64:128, dy, :],
                                 mov(ao_, dy, 1, lo=False),
                                 start=False, stop=(dy == 2),
                                 skip_group_check=True)

            # PSUM evacuation on ScalarE with bias folding
            nc.scalar.activation(qs_[64:128, j * BN:(j + 1) * BN],
                                 kqE[64:128, :], ACT.Identity,
                                 bias=bias_t[64:128, 0:1])
            nc.scalar.activation(qs_[0:64, j * BN:(j + 1) * BN],
                                 kqO[0:64, :], ACT.Identity,
                                 bias=bias_t[0:64, 0:1])
            vsb = vpool.tile([128, BN], F32, tag="vsb", name="vsb")
            nc.scalar.activation(vsb[:, :], V[:, :], ACT.Identity,
                                 bias=bias_t[:, 1:2],
                                 accum_out=vsum[pb][:, j:j + 1])

            # fused (gamma*k)*v' multiply + pixel-sum on DVE
            nc.vector.scalar_tensor_tensor(
                out=scr[0:64, :], in0=kqE[0:64, :], scalar=bias_t[0:64, 3:4],
                in1=vsb[0:64, :], op0=ALU.mult, op1=ALU.mult,
                accum_out=sparts[pb][0:64, j:j + 1])
            if kvo_mode == "stt":
                nc.vector.scalar_tensor_tensor(
                    out=scr[64:128, :], in0=kqO[64:128, :],
                    scalar=bias_t[64:128, 3:4],
                    in1=vsb[64:128, :], op0=ALU.mult, op1=ALU.mult,
                    accum_out=sparts[pb][64:128, j:j + 1])
            else:
                nc.vector.scalar_tensor_tensor(
                    out=scr[64:128, :], in0=kqO[64:128, :],
                    scalar=bias_t[64:128, 3:4],
                    in1=vsb[64:128, :], op0=ALU.mult, op1=ALU.mult)
                nc.vector.reduce_sum(sparts[pb][64:128, j:j + 1],
                                     scr[64:128, :],
                                     axis=mybir.AxisListType.X)

        # s = gamma*sum(k*v') + gamma*bk*sum(v'), then swap halves so the
        # even-slice s reaches partitions 64-127 (q/x-plain live there)
        nc.vector.reduce_sum(t1[pb][:, :], sparts[pb][:, :],
                             axis=mybir.AxisListType.X)
        nc.vector.reduce_sum(vs1[pb][:, :], vsum[pb][:, :],
                             axis=mybir.AxisListType.X)
        nc.vector.scalar_tensor_tensor(
            out=sg[pb][:, :], in0=vs1[pb][:, :], scalar=bias_t[:, 2:3],
            in1=t1[pb][:, :], op0=ALU.mult, op1=ALU.add)
        nc.scalar.dma_start(sf[pb][64:128, :], sg[pb][0:64, :])
        nc.scalar.dma_start(sf[pb][0:64, :], sg[pb][64:128, :])

        # out = q*(gamma*s) + x on Pool engine, chunked, store per chunk
        cw = W // FINCH
        cn = cw * H
        for c_ in range(FINCH):
            fin_eng.scalar_tensor_tensor(
                out=ot_[64:128, c_ * cn:(c_ + 1) * cn],
                in0=qs_[64:128, c_ * cn:(c_ + 1) * cn],
                scalar=sf[pb][64:128, 0:1],
                in1=ae_[64:128, 1 + c_ * cw:1 + (c_ + 1) * cw, 1:1 + H],
                op0=ALU.mult, op1=ALU.add)
            nc.gpsimd.dma_start(out_d[:, 2 * p, c_ * cw:(c_ + 1) * cw, :],
                                ot_[64:128, c_ * cn:(c_ + 1) * cn])
            fin_eng.scalar_tensor_tensor(
                out=ot_[0:64, c_ * cn:(c_ + 1) * cn],
                in0=qs_[0:64, c_ * cn:(c_ + 1) * cn],
                scalar=sf[pb][0:64, 0:1],
                in1=ao_[0:64, 1 + c_ * cw:1 + (c_ + 1) * cw, 1:1 + H],
                op0=ALU.mult, op1=ALU.add)
            nc.gpsimd.dma_start(out_d[:, 2 * p + 1, c_ * cw:(c_ + 1) * cw, :],
                                ot_[0:64, c_ * cn:(c_ + 1) * cn])


_CACHE = {}


def _build():
    if "nc" in _CACHE:
        return _CACHE["nc"]
    nc = bacc.Bacc("TRN2", target_bir_lowering=False, debug=False,
                   enable_asserts=False, num_devices=8)
    x_d = nc.dram_tensor("x", (C, T, W, H), BF16, kind="ExternalInput").ap()
    wkqp_d = nc.dram_tensor("wkqp", (128, 2, 3, 128), BF16,
                            kind="ExternalInput").ap()
    wkqs_d = nc.dram_tensor("wkqs", (128, 3, 128), BF16,
                            kind="ExternalInput").ap()
    wvp_d = nc.dram_tensor("wvp", (128, 2, 3, 64), BF16,
                           kind="ExternalInput").ap()
    wvs_d = nc.dram_tensor("wvs", (128, 3, 64), BF16,
                           kind="ExternalInput").ap()
    bias_d = nc.dram_tensor("bias", (128, 4), F32, kind="ExternalInput").ap()
    out_d = nc.dram_tensor("out", (C, T, W, H), BF16,
                           kind="ExternalOutput").ap()
    from contextlib import ExitStack
    with tile.TileContext(nc) as tc, ExitStack() as ctx:
        _emit(nc, tc, x_d, wkqp_d, wkqs_d, wvp_d, wvs_d, bias_d, out_d, ctx)
    nc.compile()
    _CACHE["nc"] = nc
    return nc


def run_spmd(x, wq, wk, wv, bq, bk, bv, gamma, trace=False, **kw):
    nc = _build()
    wkqp, wkqs, wvp, wvs = _pack_weights(
        np.asarray(wq, np.float32), np.asarray(wk, np.float32),
        np.asarray(wv, np.float32))
    # stationary tiles are [128(K), ...free...]: transpose packed
    # [..., K, M] so K is the partition dim
    wkqp = np.ascontiguousarray(wkqp.transpose(2, 0, 1, 3))   # [128,2,3,128]
    wkqs = np.ascontiguousarray(wkqs.transpose(1, 0, 2))      # [128,3,128]
    wvp = np.ascontiguousarray(wvp.transpose(2, 0, 1, 3))     # [128,2,3,64]
    wvs = np.ascontiguousarray(wvs.transpose(1, 0, 2))        # [128,3,64]

    gam = np.float32(np.asarray(gamma).reshape(-1)[0])
    bias = np.zeros((128, 4), np.float32)
    bias[0:64, 0] = bias[64:128, 0] = np.asarray(bq, np.float32)
    bias[0:64, 1] = bias[64:128, 1] = np.asarray(bv, np.float32)
    bias[0:64, 2] = bias[64:128, 2] = np.asarray(bk, np.float32) * gam
    bias[:, 3] = gam

    xb = _bf16(x)
    in_maps = [
        {"x": np.ascontiguousarray(xb[b]), "wkqp": wkqp, "wkqs": wkqs,
         "wvp": wvp, "wvs": wvs, "bias": bias}
        for b in range(B)
    ]
    res = bass_utils.run_bass_kernel_spmd(
        nc, in_maps, core_ids=list(range(B)), trace=trace, **kw)
    out = np.stack([np.asarray(res.results[b]["out"]).astype(np.float32)
                    for b in range(B)], axis=0)
    return out, res


def kernel(x, wq, wk, wv, bq, bk, bv, gamma):
    out, _ = run_spmd(x, wq, wk, wv, bq, bk, bv, gamma)
    return out


# revision 16
# speedup vs baseline: 1.3541x; 1.2102x over previous
"""Trainium2 Bass kernel for conv-qkv rank-1 attention.

out = gamma * q * sum((k+bk)*(v+bv)) + x, where q,k,v are per-time-slice
3x3 convs (C=64 -> C=64) of x [B=8, C=64, T=16, W=64, H=64].

Sharding: data-parallel over B across 8 cores (1 example/core), conv
weights replicated. No cross-core communication.

Per-core schedule (v2, tap-paired bf16):
Each slice keeps TWO copies of x in one SBUF tile [128, 66, 66]:
even slices [shift | plain], odd slices [plain | shift], where "shift"
is x offset one column so that a single 128-partition moving AP delivers
two different conv taps on the two partition halves. A 3x3 conv then
costs 3 K=128 "pair" matmuls (taps (dy,0)+(dy,1)) plus 3 K=64 "single"
matmuls (taps (dy,2)); singles of even/odd slices sit on disjoint PE
row groups and run concurrently (2-way row tiling), v-chain matmuls of
even/odd slices sit on disjoint column groups (2-way col tiling).
Per block of 512 pixels and slice pair this is 15 PE slots of N=512
vs 20 in the direct scheme.

Biases never enter the PE: bq/bv are folded into the PSUM->SBUF
evacuation on ScalarE (activation bias), and bk's contribution
bk*sum(v+bv) is recovered from the v-evacuation's accum_out.
The final out = q*(gamma*s) + x runs on GpSimd (Pool) so the DVE
queue never stalls the next pair's PSUM turnaround.

All matmul operands are bf16 (hosts casts x with round-to-nearest);
PSUM accumulation stays fp32.
"""

import os

import numpy as np
import ml_dtypes

import concourse.bacc as bacc
import concourse.bass as bass
import concourse.mybir as mybir
import concourse.tile as tile
from concourse import bass_utils

F32 = mybir.dt.float32
BF16 = mybir.dt.bfloat16
ALU = mybir.AluOpType
ACT = mybir.ActivationFunctionType

B, C, T, W, H = 8, 64, 16, 64, 64
WP, HP = W + 2, H + 2          # padded slice dims
NPAIR = int(os.environ.get("BASS_NPAIR", T // 2))  # slice pairs per core
RB = 8                         # W-rows per pixel block
NBLK = W // RB                 # pixel blocks per slice
BN = RB * H                    # moving free dim per matmul (512)
NABUF = 4                      # A-tile buffers per parity (4-deep rotation)
FINCH = 2                      # final-pass chunks per slice


def _bf16(a):
    return np.asarray(a, np.float32).astype(ml_dtypes.bfloat16)


def _pack_weights(wq, wk, wv):
    """Pack stationary operands (bf16).

    Moving-data convention: a pair matmul reads the full 128-partition AP
    at (r0=j*RB+dy, dx=0): on even slices the low half (shift copy)
    delivers tap (dy,1) and the high half (plain) tap (dy,0); odd slices
    are mirrored. Single matmuls read the shift copy at dx=1 -> tap
    (dy,2): even from partitions 0-63, odd from 64-127.
    kq column layout: even [Wk | Wq] (q lands on PSUM 64:128 = the
    x-plain half), odd [Wq | Wk].
    """
    def taps(w):  # [O, I, 1, 3, 3] -> tap(dy,dx) = [I, O]
        return np.ascontiguousarray(w.reshape(C, C, 3, 3).transpose(1, 2, 3, 0),
                                    np.float32)

    q_t, k_t, v_t = taps(wq), taps(wk), taps(wv)

    kq_pair = np.zeros((2, 3, 128, 128), np.float32)
    v_pair = np.zeros((2, 3, 128, 64), np.float32)
    for dy in range(3):
        # even parity
        kq_pair[0, dy, 0:64, 0:64] = k_t[:, dy, 1]
        kq_pair[0, dy, 0:64, 64:128] = q_t[:, dy, 1]
        kq_pair[0, dy, 64:128, 0:64] = k_t[:, dy, 0]
        kq_pair[0, dy, 64:128, 64:128] = q_t[:, dy, 0]
        v_pair[0, dy, 0:64, :] = v_t[:, dy, 1]
        v_pair[0, dy, 64:128, :] = v_t[:, dy, 0]
        # odd parity
        kq_pair[1, dy, 0:64, 0:64] = q_t[:, dy, 0]
        kq_pair[1, dy, 0:64, 64:128] = k_t[:, dy, 0]
        kq_pair[1, dy, 64:128, 0:64] = q_t[:, dy, 1]
        kq_pair[1, dy, 64:128, 64:128] = k_t[:, dy, 1]
        v_pair[1, dy, 0:64, :] = v_t[:, dy, 0]
        v_pair[1, dy, 64:128, :] = v_t[:, dy, 1]

    kq_sing = np.zeros((3, 128, 128), np.float32)
    v_sing = np.zeros((3, 128, 64), np.float32)
    for dy in range(3):
        kq_sing[dy, 0:64, 0:64] = k_t[:, dy, 2]
        kq_sing[dy, 0:64, 64:128] = q_t[:, dy, 2]
        kq_sing[dy, 64:128, 0:64] = q_t[:, dy, 2]
        kq_sing[dy, 64:128, 64:128] = k_t[:, dy, 2]
        v_sing[dy, 0:64, :] = v_t[:, dy, 2]
        v_sing[dy, 64:128, :] = v_t[:, dy, 2]

    return _bf16(kq_pair), _bf16(kq_sing), _bf16(v_pair), _bf16(v_sing)


def _emit(nc, tc, x_d, xp_d, wkqp_d, wkqs_d, wvp_d, wvs_d, bias_d, out_d,
          ctx):
    const = ctx.enter_context(tc.tile_pool(name="const", bufs=1))
    state = ctx.enter_context(tc.tile_pool(name="state", bufs=1))
    psum = ctx.enter_context(
        tc.tile_pool(name="psum", bufs=2, space=bass.MemorySpace.PSUM))
    vpool = ctx.enter_context(tc.tile_pool(name="vpool", bufs=2))

    wkqp_t = const.tile([128, 2, 3, 128], BF16, tag="wkqp")
    wkqs_t = const.tile([128, 3, 128], BF16, tag="wkqs")
    wvp_t = const.tile([128, 2, 3, 64], BF16, tag="wvp")
    wvs_t = const.tile([128, 3, 64], BF16, tag="wvs")
    bias_t = const.tile([128, 4], F32, tag="bias")  # bq, bv, bk*gam, gam

    nc.sync.dma_start(wkqp_t[:], wkqp_d[:])
    nc.sync.dma_start(wkqs_t[:], wkqs_d[:])
    nc.sync.dma_start(wvp_t[:], wvp_d[:])
    nc.sync.dma_start(wvs_t[:], wvs_d[:])
    nc.sync.dma_start(bias_t[:], bias_d[:])

    # A tiles: [shift | plain] for even slices, [plain | shift] for odd.
    ae = [state.tile([128, WP, HP], BF16, tag=f"ae{i}", name=f"ae{i}")
          for i in range(NABUF)]
    ao = [state.tile([128, WP, HP], BF16, tag=f"ao{i}", name=f"ao{i}")
          for i in range(NABUF)]
    qs = [state.tile([128, W * H], BF16, tag=f"qs{i}", name=f"qs{i}")
          for i in range(2)]
    ot = [state.tile([128, W * H], BF16, tag=f"ot{i}", name=f"ot{i}")
          for i in range(2)]
    scr = state.tile([128, BN], F32, tag="scr")
    sparts = [state.tile([128, NBLK], F32, tag=f"sp{i}", name=f"sp{i}")
              for i in range(2)]
    vsum = [state.tile([128, NBLK], F32, tag=f"vs{i}", name=f"vs{i}")
            for i in range(2)]
    t1 = [state.tile([128, 1], F32, tag=f"t1{i}", name=f"t1{i}")
          for i in range(2)]
    vs1 = [state.tile([128, 1], F32, tag=f"vs1{i}", name=f"vs1{i}")
           for i in range(2)]
    sg = [state.tile([128, 1], F32, tag=f"sg{i}", name=f"sg{i}")
          for i in range(2)]
    sf = [state.tile([128, 1], F32, tag=f"sf{i}", name=f"sf{i}")
          for i in range(2)]

    # plain-x tile for the full-width final pass: odd slice on partitions
    # 0-63, even slice on 64-127 (matches q/s/out halves)
    xpl = [state.tile([128, W * H], BF16, tag=f"xpl{i}", name=f"xpl{i}")
           for i in range(NABUF)]

    def load_pair(p):
        # host pre-padded slices: one contiguous descriptor per partition
        nc.sync.dma_start(ae[p % NABUF][:], xp_d[2 * p])
        nc.gpsimd.dma_start(ao[p % NABUF][:], xp_d[2 * p + 1])
        nc.scalar.dma_start(xpl[p % NABUF][0:64, :], x_d[:, 2 * p + 1])
        nc.scalar.dma_start(xpl[p % NABUF][64:128, :], x_d[:, 2 * p])

    load_pair(0)
    if NPAIR > 1:
        load_pair(1)

    kvo_mode = os.environ.get("BASS_KVO", "stt")
    fin_eng = nc.vector  # Pool lacks TensorScalarPtr; bf16 gives 2x DVE rate

    for p in range(NPAIR):
        pb = p % 2
        ae_, ao_ = ae[p % NABUF], ao[p % NABUF]
        qs_, ot_ = qs[pb], ot[pb]

        if p + 2 < NPAIR:
            load_pair(p + 2)

        for j in range(NBLK):
            kqE = psum.tile([128, BN], F32, tag="kqE", name="kqE")
            kqO = psum.tile([128, BN], F32, tag="kqO", name="kqO")
            V = psum.tile([128, BN], F32, tag="V", name="V")

            def mov(tl, dy, dx, lo=None):
                r0 = j * RB + dy
                if lo is None:
                    return tl[:, r0:r0 + RB, dx:dx + H]
                if lo:
                    return tl[0:64, r0:r0 + RB, dx:dx + H]
                return tl[64:128, r0:r0 + RB, dx:dx + H]

            # kq pair taps: K=128, full array, serial
            for dy in range(3):
                nc.tensor.matmul(kqE[:, :], wkqp_t[:, 0, dy, :],
                                 mov(ae_, dy, 0), start=(dy == 0), stop=False)
                nc.tensor.matmul(kqO[:, :], wkqp_t[:, 1, dy, :],
                                 mov(ao_, dy, 0), start=(dy == 0), stop=False)
            # kq single taps: K=64, even rows 0-63 / odd rows 64-127,
            # 2-way row-tiled concurrent
            for dy in range(3):
                nc.tensor.matmul(kqE[:, :], wkqs_t[0:64, dy, :],
                                 mov(ae_, dy, 1, lo=True),
                                 start=False, stop=(dy == 2))
                nc.tensor.matmul(kqO[:, :], wkqs_t[64:128, dy, :],
                                 mov(ao_, dy, 1, lo=False),
                                 start=False, stop=(dy == 2))
            # v pair taps: K=128 M=64, even cols 0-63 / odd cols 64-127,
            # 2-way col-tiled concurrent
            for dy in range(3):
                nc.tensor.matmul(V[0:64, :], wvp_t[:, 0, dy, :],
                                 mov(ae_, dy, 0), start=(dy == 0), stop=False,
                                 skip_group_check=True)
                nc.tensor.matmul(V[64:128, :], wvp_t[:, 1, dy, :],
                                 mov(ao_, dy, 0), start=(dy == 0), stop=False,
                                 skip_group_check=True)
            # v single taps: K=64 M=64, quadrants (0,0) and (64,64)
            for dy in range(3):
                nc.tensor.matmul(V[0:64, :], wvs_t[0:64, dy, :],
                                 mov(ae_, dy, 1, lo=True),
                                 start=False, stop=(dy == 2),
                                 skip_group_check=True)
                nc.tensor.matmul(V[64:128, :], wvs_t[64:128, dy, :],
                                 mov(ao_, dy, 1, lo=False),
                                 start=False, stop=(dy == 2),
                                 skip_group_check=True)

            # PSUM evacuation on ScalarE with bias folding
            nc.scalar.activation(qs_[64:128, j * BN:(j + 1) * BN],
                                 kqE[64:128, :], ACT.Identity,
                                 bias=bias_t[64:128, 0:1])
            nc.scalar.activation(qs_[0:64, j * BN:(j + 1) * BN],
                                 kqO[0:64, :], ACT.Identity,
                                 bias=bias_t[0:64, 0:1])
            vsb = vpool.tile([128, BN], F32, tag="vsb", name="vsb")
            nc.scalar.activation(vsb[:, :], V[:, :], ACT.Identity,
                                 bias=bias_t[:, 1:2],
                                 accum_out=vsum[pb][:, j:j + 1])

            # fused (gamma*k)*v' multiply + pixel-sum on DVE
            nc.vector.scalar_tensor_tensor(
                out=scr[0:64, :], in0=kqE[0:64, :], scalar=bias_t[0:64, 3:4],
                in1=vsb[0:64, :], op0=ALU.mult, op1=ALU.mult,
                accum_out=sparts[pb][0:64, j:j + 1])
            if kvo_mode == "stt":
                nc.vector.scalar_tensor_tensor(
                    out=scr[64:128, :], in0=kqO[64:128, :],
                    scalar=bias_t[64:128, 3:4],
                    in1=vsb[64:128, :], op0=ALU.mult, op1=ALU.mult,
                    accum_out=sparts[pb][64:128, j:j + 1])
            else:
                nc.vector.scalar_tensor_tensor(
                    out=scr[64:128, :], in0=kqO[64:128, :],
                    scalar=bias_t[64:128, 3:4],
                    in1=vsb[64:128, :], op0=ALU.mult, op1=ALU.mult)
                nc.vector.reduce_sum(sparts[pb][64:128, j:j + 1],
                                     scr[64:128, :],
                                     axis=mybir.AxisListType.X)

        # s = gamma*sum(k*v') + gamma*bk*sum(v'), then swap halves so the
        # even-slice s reaches partitions 64-127 (q/x-plain live there)
        nc.vector.reduce_sum(t1[pb][:, :], sparts[pb][:, :],
                             axis=mybir.AxisListType.X)
        nc.vector.reduce_sum(vs1[pb][:, :], vsum[pb][:, :],
                             axis=mybir.AxisListType.X)
        nc.vector.scalar_tensor_tensor(
            out=sg[pb][:, :], in0=vs1[pb][:, :], scalar=bias_t[:, 2:3],
            in1=t1[pb][:, :], op0=ALU.mult, op1=ALU.add)
        nc.scalar.dma_start(sf[pb][64:128, :], sg[pb][0:64, :])
        nc.scalar.dma_start(sf[pb][0:64, :], sg[pb][64:128, :])

        # out = q*(gamma*s) + x, full 128-partition DVE ops (both slices at
        # once; xpl interleaves odd/even plain x to match), store per chunk
        cw = W // FINCH
        cn = cw * H
        for c_ in range(FINCH):
            fin_eng.scalar_tensor_tensor(
                out=ot_[:, c_ * cn:(c_ + 1) * cn],
                in0=qs_[:, c_ * cn:(c_ + 1) * cn],
                scalar=sf[pb][:, 0:1],
                in1=xpl[p % NABUF][:, c_ * cn:(c_ + 1) * cn],
                op0=ALU.mult, op1=ALU.add)
            nc.gpsimd.dma_start(out_d[:, 2 * p, c_ * cw:(c_ + 1) * cw, :],
                                ot_[64:128, c_ * cn:(c_ + 1) * cn])
            nc.gpsimd.dma_start(out_d[:, 2 * p + 1, c_ * cw:(c_ + 1) * cw, :],
                                ot_[0:64, c_ * cn:(c_ + 1) * cn])


_CACHE = {}


def _build():
    if "nc" in _CACHE:
        return _CACHE["nc"]
    nc = bacc.Bacc("TRN2", target_bir_lowering=False, debug=False,
                   enable_asserts=False, num_devices=8)
    x_d = nc.dram_tensor("x", (C, T, W, H), BF16, kind="ExternalInput").ap()
    xp_d = nc.dram_tensor("xp", (T, 128, WP, HP), BF16,
                          kind="ExternalInput").ap()
    wkqp_d = nc.dram_tensor("wkqp", (128, 2, 3, 128), BF16,
                            kind="ExternalInput").ap()
    wkqs_d = nc.dram_tensor("wkqs", (128, 3, 128), BF16,
                            kind="ExternalInput").ap()
    wvp_d = nc.dram_tensor("wvp", (128, 2, 3, 64), BF16,
                           kind="ExternalInput").ap()
    wvs_d = nc.dram_tensor("wvs", (128, 3, 64), BF16,
                           kind="ExternalInput").ap()
    bias_d = nc.dram_tensor("bias", (128, 4), F32, kind="ExternalInput").ap()
    out_d = nc.dram_tensor("out", (C, T, W, H), BF16,
                           kind="ExternalOutput").ap()
    from contextlib import ExitStack
    with tile.TileContext(nc) as tc, ExitStack() as ctx:
        _emit(nc, tc, x_d, xp_d, wkqp_d, wkqs_d, wvp_d, wvs_d, bias_d, out_d,
              ctx)
    nc.compile()
    _CACHE["nc"] = nc
    return nc


def run_spmd(x, wq, wk, wv, bq, bk, bv, gamma, trace=False, **kw):
    nc = _build()
    wkqp, wkqs, wvp, wvs = _pack_weights(
        np.asarray(wq, np.float32), np.asarray(wk, np.float32),
        np.asarray(wv, np.float32))
    # stationary tiles are [128(K), ...free...]: transpose packed
    # [..., K, M] so K is the partition dim
    wkqp = np.ascontiguousarray(wkqp.transpose(2, 0, 1, 3))   # [128,2,3,128]
    wkqs = np.ascontiguousarray(wkqs.transpose(1, 0, 2))      # [128,3,128]
    wvp = np.ascontiguousarray(wvp.transpose(2, 0, 1, 3))     # [128,2,3,64]
    wvs = np.ascontiguousarray(wvs.transpose(1, 0, 2))        # [128,3,64]

    gam = np.float32(np.asarray(gamma).reshape(-1)[0])
    bias = np.zeros((128, 4), np.float32)
    bias[0:64, 0] = bias[64:128, 0] = np.asarray(bq, np.float32)
    bias[0:64, 1] = bias[64:128, 1] = np.asarray(bv, np.float32)
    bias[0:64, 2] = bias[64:128, 2] = np.asarray(bk, np.float32) * gam
    bias[:, 3] = gam

    xb = _bf16(x)
    # host pre-padded per-slice layout [T, 128, WP, HP]: even slices
    # [shift | plain], odd slices [plain | shift] on the partition halves
    zpad = np.zeros((B, T, C, WP, HP), ml_dtypes.bfloat16)
    zsh = np.zeros((B, T, C, WP, HP), ml_dtypes.bfloat16)
    xt = xb.transpose(0, 2, 1, 3, 4)            # [B, T, C, W, H]
    zpad[:, :, :, 1:1 + W, 1:1 + H] = xt
    zsh[:, :, :, 1:1 + W, 0:H] = xt
    xp = np.empty((B, T, 128, WP, HP), ml_dtypes.bfloat16)
    xp[:, 0::2, 0:64] = zsh[:, 0::2]
    xp[:, 0::2, 64:128] = zpad[:, 0::2]
    xp[:, 1::2, 0:64] = zpad[:, 1::2]
    xp[:, 1::2, 64:128] = zsh[:, 1::2]
    in_maps = [
        {"x": np.ascontiguousarray(xb[b]), "xp": np.ascontiguousarray(xp[b]),
         "wkqp": wkqp, "wkqs": wkqs,
         "wvp": wvp, "wvs": wvs, "bias": bias}
        for b in range(B)
    ]
    res = bass_utils.run_bass_kernel_spmd(
        nc, in_maps, core_ids=list(range(B)), trace=trace, **kw)
    out = np.stack([np.asarray(res.results[b]["out"]).astype(np.float32)
                    for b in range(B)], axis=0)
    return out, res


def kernel(x, wq, wk, wv, bq, bk, bv, gamma):
    out, _ = run_spmd(x, wq, wk, wv, bq, bk, bv, gamma)
    return out


# revision 20
# speedup vs baseline: 1.4832x; 1.0953x over previous
"""Trainium2 Bass kernel for conv-qkv rank-1 attention.

out = gamma * q * sum((k+bk)*(v+bv)) + x, where q,k,v are per-time-slice
3x3 convs (C=64 -> C=64) of x [B=8, C=64, T=16, W=64, H=64].

Sharding: data-parallel over B across 8 cores (1 example/core), conv
weights replicated. No cross-core communication.

Per-core schedule (v2, tap-paired bf16):
Each slice keeps TWO copies of x in one SBUF tile [128, 66, 66]:
even slices [shift | plain], odd slices [plain | shift], where "shift"
is x offset one column so that a single 128-partition moving AP delivers
two different conv taps on the two partition halves. A 3x3 conv then
costs 3 K=128 "pair" matmuls (taps (dy,0)+(dy,1)) plus 3 K=64 "single"
matmuls (taps (dy,2)); singles of even/odd slices sit on disjoint PE
row groups and run concurrently (2-way row tiling), v-chain matmuls of
even/odd slices sit on disjoint column groups (2-way col tiling).
Per block of 512 pixels and slice pair this is 15 PE slots of N=512
vs 20 in the direct scheme.

Biases never enter the PE: bq/bv are folded into the PSUM->SBUF
evacuation on ScalarE (activation bias), and bk's contribution
bk*sum(v+bv) is recovered from the v-evacuation's accum_out.
The final out = q*(gamma*s) + x runs on GpSimd (Pool) so the DVE
queue never stalls the next pair's PSUM turnaround.

All matmul operands are bf16 (hosts casts x with round-to-nearest);
PSUM accumulation stays fp32.
"""

import os

import numpy as np
import ml_dtypes

import concourse.bacc as bacc
import concourse.bass as bass
import concourse.mybir as mybir
import concourse.tile as tile
from concourse import bass_utils

F32 = mybir.dt.float32
BF16 = mybir.dt.bfloat16
ALU = mybir.AluOpType
ACT = mybir.ActivationFunctionType

B, C, T, W, H = 8, 64, 16, 64, 64
WP, HP = W + 2, H + 2          # padded slice dims
NPAIR = int(os.environ.get("BASS_NPAIR", T // 2))  # slice pairs per core
RB = 8                         # W-rows per pixel block
NBLK = W // RB                 # pixel blocks per slice
BN = RB * H                    # moving free dim per matmul (512)
NABUF = 4                      # A-tile buffers per parity (4-deep rotation)
FINCH = 2                      # final-pass chunks per slice


def _bf16(a):
    return np.asarray(a, np.float32).astype(ml_dtypes.bfloat16)


def _pack_weights(wq, wk, wv):
    """Pack stationary operands (bf16).

    Moving-data convention: a pair matmul reads the full 128-partition AP
    at (r0=j*RB+dy, dx=0): on even slices the low half (shift copy)
    delivers tap (dy,1) and the high half (plain) tap (dy,0); odd slices
    are mirrored. Single matmuls read the shift copy at dx=1 -> tap
    (dy,2): even from partitions 0-63, odd from 64-127.
    kq column layout: even [Wk | Wq] (q lands on PSUM 64:128 = the
    x-plain half), odd [Wq | Wk].
    """
    def taps(w):  # [O, I, 1, 3, 3] -> tap(dy,dx) = [I, O]
        return np.ascontiguousarray(w.reshape(C, C, 3, 3).transpose(1, 2, 3, 0),
                                    np.float32)

    q_t, k_t, v_t = taps(wq), taps(wk), taps(wv)

    kq_pair = np.zeros((2, 3, 128, 128), np.float32)
    v_pair = np.zeros((2, 3, 128, 64), np.float32)
    for dy in range(3):
        # even parity
        kq_pair[0, dy, 0:64, 0:64] = k_t[:, dy, 1]
        kq_pair[0, dy, 0:64, 64:128] = q_t[:, dy, 1]
        kq_pair[0, dy, 64:128, 0:64] = k_t[:, dy, 0]
        kq_pair[0, dy, 64:128, 64:128] = q_t[:, dy, 0]
        v_pair[0, dy, 0:64, :] = v_t[:, dy, 1]
        v_pair[0, dy, 64:128, :] = v_t[:, dy, 0]
        # odd parity
        kq_pair[1, dy, 0:64, 0:64] = q_t[:, dy, 0]
        kq_pair[1, dy, 0:64, 64:128] = k_t[:, dy, 0]
        kq_pair[1, dy, 64:128, 0:64] = q_t[:, dy, 1]
        kq_pair[1, dy, 64:128, 64:128] = k_t[:, dy, 1]
        v_pair[1, dy, 0:64, :] = v_t[:, dy, 0]
        v_pair[1, dy, 64:128, :] = v_t[:, dy, 1]

    kq_sing = np.zeros((3, 128, 128), np.float32)
    v_sing = np.zeros((3, 128, 64), np.float32)
    for dy in range(3):
        kq_sing[dy, 0:64, 0:64] = k_t[:, dy, 2]
        kq_sing[dy, 0:64, 64:128] = q_t[:, dy, 2]
        kq_sing[dy, 64:128, 0:64] = q_t[:, dy, 2]
        kq_sing[dy, 64:128, 64:128] = k_t[:, dy, 2]
        v_sing[dy, 0:64, :] = v_t[:, dy, 2]
        v_sing[dy, 64:128, :] = v_t[:, dy, 2]

    return _bf16(kq_pair), _bf16(kq_sing), _bf16(v_pair), _bf16(v_sing)


def _emit(nc, tc, x_d, xp_d, wkqp_d, wkqs_d, wvp_d, wvs_d, bias_d, out_d,
          ctx):
    const = ctx.enter_context(tc.tile_pool(name="const", bufs=1))
    state = ctx.enter_context(tc.tile_pool(name="state", bufs=1))
    psum = ctx.enter_context(
        tc.tile_pool(name="psum", bufs=2, space=bass.MemorySpace.PSUM))
    vpool = ctx.enter_context(tc.tile_pool(name="vpool", bufs=2))

    wkqp_t = const.tile([128, 2, 3, 128], BF16, tag="wkqp")
    wkqs_t = const.tile([128, 3, 128], BF16, tag="wkqs")
    wvp_t = const.tile([128, 2, 3, 64], BF16, tag="wvp")
    wvs_t = const.tile([128, 3, 64], BF16, tag="wvs")
    bias_t = const.tile([128, 4], F32, tag="bias")  # bq, bv, bk*gam, gam

    nc.sync.dma_start(wkqp_t[:], wkqp_d[:])
    nc.sync.dma_start(wkqs_t[:], wkqs_d[:])
    nc.sync.dma_start(wvp_t[:], wvp_d[:])
    nc.sync.dma_start(wvs_t[:], wvs_d[:])
    nc.sync.dma_start(bias_t[:], bias_d[:])

    # A tiles: [shift | plain] for even slices, [plain | shift] for odd.
    ae = [state.tile([128, WP, HP], BF16, tag=f"ae{i}", name=f"ae{i}")
          for i in range(NABUF)]
    ao = [state.tile([128, WP, HP], BF16, tag=f"ao{i}", name=f"ao{i}")
          for i in range(NABUF)]
    qs = [state.tile([128, W * H], BF16, tag=f"qs{i}", name=f"qs{i}")
          for i in range(2)]
    ot = [state.tile([128, W * H], BF16, tag=f"ot{i}", name=f"ot{i}")
          for i in range(2)]
    scr = state.tile([128, BN], F32, tag="scr")
    sparts = [state.tile([128, NBLK], F32, tag=f"sp{i}", name=f"sp{i}")
              for i in range(2)]
    vsum = [state.tile([128, NBLK], F32, tag=f"vs{i}", name=f"vs{i}")
            for i in range(2)]
    t1 = [state.tile([128, 1], F32, tag=f"t1{i}", name=f"t1{i}")
          for i in range(2)]
    vs1 = [state.tile([128, 1], F32, tag=f"vs1{i}", name=f"vs1{i}")
           for i in range(2)]
    sg = [state.tile([128, 1], F32, tag=f"sg{i}", name=f"sg{i}")
          for i in range(2)]
    sf = [state.tile([128, 1], F32, tag=f"sf{i}", name=f"sf{i}")
          for i in range(2)]

    # plain-x tile for the full-width final pass: odd slice on partitions
    # 0-63, even slice on 64-127 (matches q/s/out halves)
    xpl = [state.tile([128, W * H], BF16, tag=f"xpl{i}", name=f"xpl{i}")
           for i in range(NABUF)]

    def load_pair(p):
        # host pre-padded slices: one contiguous descriptor per partition
        nc.sync.dma_start(ae[p % NABUF][:], xp_d[2 * p])
        nc.gpsimd.dma_start(ao[p % NABUF][:], xp_d[2 * p + 1])
        nc.sync.dma_start(xpl[p % NABUF][0:64, :], x_d[:, 2 * p + 1])
        nc.sync.dma_start(xpl[p % NABUF][64:128, :], x_d[:, 2 * p])

    load_pair(0)
    if NPAIR > 1:
        load_pair(1)

    kvo_mode = os.environ.get("BASS_KVO", "stt")
    fin_eng = nc.vector  # Pool lacks TensorScalarPtr; bf16 gives 2x DVE rate

    def emit_final(p):
        # out = q*(gamma*s) + x, full 128-partition DVE ops (both slices
        # at once; xpl interleaves odd/even plain x), store per chunk
        pb = p % 2
        cw = W // FINCH
        cn = cw * H
        for c_ in range(FINCH):
            fin_eng.scalar_tensor_tensor(
                out=ot[pb][:, c_ * cn:(c_ + 1) * cn],
                in0=qs[pb][:, c_ * cn:(c_ + 1) * cn],
                scalar=sf[pb][:, 0:1],
                in1=xpl[p % NABUF][:, c_ * cn:(c_ + 1) * cn],
                op0=ALU.mult, op1=ALU.add)
            nc.gpsimd.dma_start(out_d[:, 2 * p, c_ * cw:(c_ + 1) * cw, :],
                                ot[pb][64:128, c_ * cn:(c_ + 1) * cn])
            nc.gpsimd.dma_start(out_d[:, 2 * p + 1, c_ * cw:(c_ + 1) * cw, :],
                                ot[pb][0:64, c_ * cn:(c_ + 1) * cn])

    for p in range(NPAIR):
        pb = p % 2
        ae_, ao_ = ae[p % NABUF], ao[p % NABUF]
        qs_, ot_ = qs[pb], ot[pb]

        for j in range(NBLK):
            kqE = psum.tile([128, BN], F32, tag="kqE", name="kqE")
            kqO = psum.tile([128, BN], F32, tag="kqO", name="kqO")
            V = psum.tile([128, BN], F32, tag="V", name="V")

            def mov(tl, dy, dx, lo=None):
                r0 = j * RB + dy
                if lo is None:
                    return tl[:, r0:r0 + RB, dx:dx + H]
                if lo:
                    return tl[0:64, r0:r0 + RB, dx:dx + H]
                return tl[64:128, r0:r0 + RB, dx:dx + H]

            # kq pair taps: K=128, full array, serial
            for dy in range(3):
                nc.tensor.matmul(kqE[:, :], wkqp_t[:, 0, dy, :],
                                 mov(ae_, dy, 0), start=(dy == 0), stop=False)
                nc.tensor.matmul(kqO[:, :], wkqp_t[:, 1, dy, :],
                                 mov(ao_, dy, 0), start=(dy == 0), stop=False)
            # kq single taps: K=64, even rows 0-63 / odd rows 64-127,
            # 2-way row-tiled concurrent
            for dy in range(3):
                nc.tensor.matmul(kqE[:, :], wkqs_t[0:64, dy, :],
                                 mov(ae_, dy, 1, lo=True),
                                 start=False, stop=(dy == 2))
                nc.tensor.matmul(kqO[:, :], wkqs_t[64:128, dy, :],
                                 mov(ao_, dy, 1, lo=False),
                                 start=False, stop=(dy == 2))
            # v pair taps: K=128 M=64, even cols 0-63 / odd cols 64-127,
            # 2-way col-tiled concurrent
            for dy in range(3):
                nc.tensor.matmul(V[0:64, :], wvp_t[:, 0, dy, :],
                                 mov(ae_, dy, 0), start=(dy == 0), stop=False,
                                 skip_group_check=True)
                nc.tensor.matmul(V[64:128, :], wvp_t[:, 1, dy, :],
                                 mov(ao_, dy, 0), start=(dy == 0), stop=False,
                                 skip_group_check=True)
            # v single taps: K=64 M=64, quadrants (0,0) and (64,64)
            for dy in range(3):
                nc.tensor.matmul(V[0:64, :], wvs_t[0:64, dy, :],
                                 mov(ae_, dy, 1, lo=True),
                                 start=False, stop=(dy == 2),
                                 skip_group_check=True)
                nc.tensor.matmul(V[64:128, :], wvs_t[64:128, dy, :],
                                 mov(ao_, dy, 1, lo=False),
                                 start=False, stop=(dy == 2),
                                 skip_group_check=True)

            # PSUM evacuation on ScalarE with bias folding
            nc.scalar.activation(qs_[64:128, j * BN:(j + 1) * BN],
                                 kqE[64:128, :], ACT.Identity,
                                 bias=bias_t[64:128, 0:1])
            nc.scalar.activation(qs_[0:64, j * BN:(j + 1) * BN],
                                 kqO[0:64, :], ACT.Identity,
                                 bias=bias_t[0:64, 0:1])
            vsb = vpool.tile([128, BN], F32, tag="vsb", name="vsb")
            nc.scalar.activation(vsb[:, :], V[:, :], ACT.Identity,
                                 bias=bias_t[:, 1:2],
                                 accum_out=vsum[pb][:, j:j + 1])

            # fused (gamma*k)*v' multiply + pixel-sum on DVE
            nc.vector.scalar_tensor_tensor(
                out=scr[0:64, :], in0=kqE[0:64, :], scalar=bias_t[0:64, 3:4],
                in1=vsb[0:64, :], op0=ALU.mult, op1=ALU.mult,
                accum_out=sparts[pb][0:64, j:j + 1])
            if kvo_mode == "stt":
                nc.vector.scalar_tensor_tensor(
                    out=scr[64:128, :], in0=kqO[64:128, :],
                    scalar=bias_t[64:128, 3:4],
                    in1=vsb[64:128, :], op0=ALU.mult, op1=ALU.mult,
                    accum_out=sparts[pb][64:128, j:j + 1])
            else:
                nc.vector.scalar_tensor_tensor(
                    out=scr[64:128, :], in0=kqO[64:128, :],
                    scalar=bias_t[64:128, 3:4],
                    in1=vsb[64:128, :], op0=ALU.mult, op1=ALU.mult)
                nc.vector.reduce_sum(sparts[pb][64:128, j:j + 1],
                                     scr[64:128, :],
                                     axis=mybir.AxisListType.X)

            # previous pair's final pass, deferred so the DVE queue never
            # holds this pair's psum turnaround behind it; prefetch loads
            # likewise emitted mid-pair
            if j == 1 and p > 0:
                emit_final(p - 1)
            if j == 3 and p + 2 < NPAIR:
                load_pair(p + 2)

        # s = gamma*sum(k*v') + gamma*bk*sum(v'), then swap halves so the
        # even-slice s reaches partitions 64-127 (q/x-plain live there)
        nc.vector.reduce_sum(t1[pb][:, :], sparts[pb][:, :],
                             axis=mybir.AxisListType.X)
        nc.vector.reduce_sum(vs1[pb][:, :], vsum[pb][:, :],
                             axis=mybir.AxisListType.X)
        nc.vector.scalar_tensor_tensor(
            out=sg[pb][:, :], in0=vs1[pb][:, :], scalar=bias_t[:, 2:3],
            in1=t1[pb][:, :], op0=ALU.mult, op1=ALU.add)
        nc.scalar.dma_start(sf[pb][64:128, :], sg[pb][0:64, :])
        nc.scalar.dma_start(sf[pb][0:64, :], sg[pb][64:128, :])

    emit_final(NPAIR - 1)


_CACHE = {}


def _build():
    if "nc" in _CACHE:
        return _CACHE["nc"]
    nc = bacc.Bacc("TRN2", target_bir_lowering=False, debug=False,
                   enable_asserts=False, num_devices=8)
    x_d = nc.dram_tensor("x", (C, T, W, H), BF16, kind="ExternalInput").ap()
    xp_d = nc.dram_tensor("xp", (T, 128, WP, HP), BF16,
                          kind="ExternalInput").ap()
    wkqp_d = nc.dram_tensor("wkqp", (128, 2, 3, 128), BF16,
                            kind="ExternalInput").ap()
    wkqs_d = nc.dram_tensor("wkqs", (128, 3, 128), BF16,
                            kind="ExternalInput").ap()
    wvp_d = nc.dram_tensor("wvp", (128, 2, 3, 64), BF16,
                           kind="ExternalInput").ap()
    wvs_d = nc.dram_tensor("wvs", (128, 3, 64), BF16,
                           kind="ExternalInput").ap()
    bias_d = nc.dram_tensor("bias", (128, 4), F32, kind="ExternalInput").ap()
    out_d = nc.dram_tensor("out", (C, T, W, H), BF16,
                           kind="ExternalOutput").ap()
    from contextlib import ExitStack
    with tile.TileContext(nc) as tc, ExitStack() as ctx:
        _emit(nc, tc, x_d, xp_d, wkqp_d, wkqs_d, wvp_d, wvs_d, bias_d, out_d,
              ctx)
    nc.compile()
    _CACHE["nc"] = nc
    return nc


def run_spmd(x, wq, wk, wv, bq, bk, bv, gamma, trace=False, **kw):
    nc = _build()
    wkqp, wkqs, wvp, wvs = _pack_weights(
        np.asarray(wq, np.float32), np.asarray(wk, np.float32),
        np.asarray(wv, np.float32))
    # stationary tiles are [128(K), ...free...]: transpose packed
    # [..., K, M] so K is the partition dim
    wkqp = np.ascontiguousarray(wkqp.transpose(2, 0, 1, 3))   # [128,2,3,128]
    wkqs = np.ascontiguousarray(wkqs.transpose(1, 0, 2))      # [128,3,128]
    wvp = np.ascontiguousarray(wvp.transpose(2, 0, 1, 3))     # [128,2,3,64]
    wvs = np.ascontiguousarray(wvs.transpose(1, 0, 2))        # [128,3,64]

    gam = np.float32(np.asarray(gamma).reshape(-1)[0])
    bias = np.zeros((128, 4), np.float32)
    bias[0:64, 0] = bias[64:128, 0] = np.asarray(bq, np.float32)
    bias[0:64, 1] = bias[64:128, 1] = np.asarray(bv, np.float32)
    bias[0:64, 2] = bias[64:128, 2] = np.asarray(bk, np.float32) * gam
    bias[:, 3] = gam

    xb = _bf16(x)
    # host pre-padded per-slice layout [T, 128, WP, HP]: even slices
    # [shift | plain], odd slices [plain | shift] on the partition halves
    zpad = np.zeros((B, T, C, WP, HP), ml_dtypes.bfloat16)
    zsh = np.zeros((B, T, C, WP, HP), ml_dtypes.bfloat16)
    xt = xb.transpose(0, 2, 1, 3, 4)            # [B, T, C, W, H]
    zpad[:, :, :, 1:1 + W, 1:1 + H] = xt
    zsh[:, :, :, 1:1 + W, 0:H] = xt
    xp = np.empty((B, T, 128, WP, HP), ml_dtypes.bfloat16)
    xp[:, 0::2, 0:64] = zsh[:, 0::2]
    xp[:, 0::2, 64:128] = zpad[:, 0::2]
    xp[:, 1::2, 0:64] = zpad[:, 1::2]
    xp[:, 1::2, 64:128] = zsh[:, 1::2]
    in_maps = [
        {"x": np.ascontiguousarray(xb[b]), "xp": np.ascontiguousarray(xp[b]),
         "wkqp": wkqp, "wkqs": wkqs,
         "wvp": wvp, "wvs": wvs, "bias": bias}
        for b in range(B)
    ]
    res = bass_utils.run_bass_kernel_spmd(
        nc, in_maps, core_ids=list(range(B)), trace=trace, **kw)
    out = np.stack([np.asarray(res.results[b]["out"]).astype(np.float32)
                    for b in range(B)], axis=0)
    return out, res


def kernel(x, wq, wk, wv, bq, bk, bv, gamma):
    out, _ = run_spmd(x, wq, wk, wv, bq, bk, bv, gamma)
    return out


# revision 23
# speedup vs baseline: 1.5889x; 1.0713x over previous
"""Trainium2 Bass kernel for conv-qkv rank-1 attention.

out = gamma * q * sum((k+bk)*(v+bv)) + x, where q,k,v are per-time-slice
3x3 convs (C=64 -> C=64) of x [B=8, C=64, T=16, W=64, H=64].

Sharding: data-parallel over B across 8 cores (1 example/core), conv
weights replicated. No cross-core communication.

Per-core schedule (v2, tap-paired bf16):
Each slice keeps TWO copies of x in one SBUF tile [128, 66, 66]:
even slices [shift | plain], odd slices [plain | shift], where "shift"
is x offset one column so that a single 128-partition moving AP delivers
two different conv taps on the two partition halves. A 3x3 conv then
costs 3 K=128 "pair" matmuls (taps (dy,0)+(dy,1)) plus 3 K=64 "single"
matmuls (taps (dy,2)); singles of even/odd slices sit on disjoint PE
row groups and run concurrently (2-way row tiling), v-chain matmuls of
even/odd slices sit on disjoint column groups (2-way col tiling).
Per block of 512 pixels and slice pair this is 15 PE slots of N=512
vs 20 in the direct scheme.

Biases never enter the PE: bq/bv are folded into the PSUM->SBUF
evacuation on ScalarE (activation bias), and bk's contribution
bk*sum(v+bv) is recovered from the v-evacuation's accum_out.
The final out = q*(gamma*s) + x runs on GpSimd (Pool) so the DVE
queue never stalls the next pair's PSUM turnaround.

All matmul operands are bf16 (hosts casts x with round-to-nearest);
PSUM accumulation stays fp32.
"""

import os

import numpy as np
import ml_dtypes

import concourse.bacc as bacc
import concourse.bass as bass
import concourse.mybir as mybir
import concourse.tile as tile
from concourse import bass_utils

F32 = mybir.dt.float32
BF16 = mybir.dt.bfloat16
ALU = mybir.AluOpType
ACT = mybir.ActivationFunctionType

B, C, T, W, H = 8, 64, 16, 64, 64
WP, HP = W + 2, H + 2          # padded slice dims
NPAIR = int(os.environ.get("BASS_NPAIR", T // 2))  # slice pairs per core
RB = 8                         # W-rows per pixel block
NBLK = W // RB                 # pixel blocks per slice
BN = RB * H                    # moving free dim per matmul (512)
NABUF = 4                      # A-tile buffers per parity (4-deep rotation)
FINCH = 4                      # final-pass chunks per slice


def _bf16(a):
    return np.asarray(a, np.float32).astype(ml_dtypes.bfloat16)


def _pack_weights(wq, wk, wv):
    """Pack stationary operands (bf16).

    Moving-data convention: a pair matmul reads the full 128-partition AP
    at (r0=j*RB+dy, dx=0): on even slices the low half (shift copy)
    delivers tap (dy,1) and the high half (plain) tap (dy,0); odd slices
    are mirrored. Single matmuls read the shift copy at dx=1 -> tap
    (dy,2): even from partitions 0-63, odd from 64-127.
    kq column layout: even [Wk | Wq] (q lands on PSUM 64:128 = the
    x-plain half), odd [Wq | Wk].
    """
    def taps(w):  # [O, I, 1, 3, 3] -> tap(dy,dx) = [I, O]
        return np.ascontiguousarray(w.reshape(C, C, 3, 3).transpose(1, 2, 3, 0),
                                    np.float32)

    q_t, k_t, v_t = taps(wq), taps(wk), taps(wv)

    kq_pair = np.zeros((2, 3, 128, 128), np.float32)
    v_pair = np.zeros((2, 3, 128, 64), np.float32)
    for dy in range(3):
        # even parity
        kq_pair[0, dy, 0:64, 0:64] = k_t[:, dy, 1]
        kq_pair[0, dy, 0:64, 64:128] = q_t[:, dy, 1]
        kq_pair[0, dy, 64:128, 0:64] = k_t[:, dy, 0]
        kq_pair[0, dy, 64:128, 64:128] = q_t[:, dy, 0]
        v_pair[0, dy, 0:64, :] = v_t[:, dy, 1]
        v_pair[0, dy, 64:128, :] = v_t[:, dy, 0]
        # odd parity
        kq_pair[1, dy, 0:64, 0:64] = q_t[:, dy, 0]
        kq_pair[1, dy, 0:64, 64:128] = k_t[:, dy, 0]
        kq_pair[1, dy, 64:128, 0:64] = q_t[:, dy, 1]
        kq_pair[1, dy, 64:128, 64:128] = k_t[:, dy, 1]
        v_pair[1, dy, 0:64, :] = v_t[:, dy, 0]
        v_pair[1, dy, 64:128, :] = v_t[:, dy, 1]

    kq_sing = np.zeros((3, 128, 128), np.float32)
    v_sing = np.zeros((3, 128, 64), np.float32)
    for dy in range(3):
        kq_sing[dy, 0:64, 0:64] = k_t[:, dy, 2]
        kq_sing[dy, 0:64, 64:128] = q_t[:, dy, 2]
        kq_sing[dy, 64:128, 0:64] = q_t[:, dy, 2]
        kq_sing[dy, 64:128, 64:128] = k_t[:, dy, 2]
        v_sing[dy, 0:64, :] = v_t[:, dy, 2]
        v_sing[dy, 64:128, :] = v_t[:, dy, 2]

    return _bf16(kq_pair), _bf16(kq_sing), _bf16(v_pair), _bf16(v_sing)


def _emit(nc, tc, x_d, xp_d, wkqp_d, wkqs_d, wvp_d, wvs_d, bias_d, out_d,
          ctx):
    const = ctx.enter_context(tc.tile_pool(name="const", bufs=1))
    state = ctx.enter_context(tc.tile_pool(name="state", bufs=1))
    psum = ctx.enter_context(
        tc.tile_pool(name="psum", bufs=3, space=bass.MemorySpace.PSUM))
    psumv = ctx.enter_context(
        tc.tile_pool(name="psumv", bufs=2, space=bass.MemorySpace.PSUM))
    vpool = ctx.enter_context(tc.tile_pool(name="vpool", bufs=2))

    wkqp_t = const.tile([128, 2, 3, 128], BF16, tag="wkqp")
    wkqs_t = const.tile([128, 3, 128], BF16, tag="wkqs")
    wvp_t = const.tile([128, 2, 3, 64], BF16, tag="wvp")
    wvs_t = const.tile([128, 3, 64], BF16, tag="wvs")
    bias_t = const.tile([128, 4], F32, tag="bias")  # bq, bv, bk*gam, gam

    nc.sync.dma_start(wkqp_t[:], wkqp_d[:])
    nc.sync.dma_start(wkqs_t[:], wkqs_d[:])
    nc.sync.dma_start(wvp_t[:], wvp_d[:])
    nc.sync.dma_start(wvs_t[:], wvs_d[:])
    nc.sync.dma_start(bias_t[:], bias_d[:])

    # A tiles: [shift | plain] for even slices, [plain | shift] for odd.
    ae = [state.tile([128, WP, HP], BF16, tag=f"ae{i}", name=f"ae{i}")
          for i in range(NABUF)]
    ao = [state.tile([128, WP, HP], BF16, tag=f"ao{i}", name=f"ao{i}")
          for i in range(NABUF)]
    qs = [state.tile([128, W * H], BF16, tag=f"qs{i}", name=f"qs{i}")
          for i in range(2)]
    ot = [state.tile([128, W * H], BF16, tag=f"ot{i}", name=f"ot{i}")
          for i in range(2)]
    scr = state.tile([128, BN], F32, tag="scr")
    sparts = [state.tile([128, NBLK], F32, tag=f"sp{i}", name=f"sp{i}")
              for i in range(2)]
    vsum = [state.tile([128, NBLK], F32, tag=f"vs{i}", name=f"vs{i}")
            for i in range(2)]
    t1 = [state.tile([128, 1], F32, tag=f"t1{i}", name=f"t1{i}")
          for i in range(2)]
    vs1 = [state.tile([128, 1], F32, tag=f"vs1{i}", name=f"vs1{i}")
           for i in range(2)]
    sg = [state.tile([128, 1], F32, tag=f"sg{i}", name=f"sg{i}")
          for i in range(2)]
    sf = [state.tile([128, 1], F32, tag=f"sf{i}", name=f"sf{i}")
          for i in range(2)]

    # plain-x tile for the full-width final pass: odd slice on partitions
    # 0-63, even slice on 64-127 (matches q/s/out halves)
    xpl = [state.tile([128, W * H], BF16, tag=f"xpl{i}", name=f"xpl{i}")
           for i in range(NABUF)]

    def load_pair(p):
        # host pre-padded slices: one contiguous descriptor per partition
        nc.sync.dma_start(ae[p % NABUF][:], xp_d[2 * p])
        nc.gpsimd.dma_start(ao[p % NABUF][:], xp_d[2 * p + 1])
        nc.sync.dma_start(xpl[p % NABUF][0:64, :], x_d[:, 2 * p + 1])
        nc.sync.dma_start(xpl[p % NABUF][64:128, :], x_d[:, 2 * p])

    load_pair(0)
    if NPAIR > 1:
        load_pair(1)

    kvo_mode = os.environ.get("BASS_KVO", "stt")
    fin_eng = nc.vector  # Pool lacks TensorScalarPtr; bf16 gives 2x DVE rate

    def emit_final(p):
        # out = q*(gamma*s) + x, full 128-partition DVE ops (both slices
        # at once; xpl interleaves odd/even plain x), store per chunk
        pb = p % 2
        cw = W // FINCH
        cn = cw * H
        for c_ in range(FINCH):
            fin_eng.scalar_tensor_tensor(
                out=ot[pb][:, c_ * cn:(c_ + 1) * cn],
                in0=qs[pb][:, c_ * cn:(c_ + 1) * cn],
                scalar=sf[pb][:, 0:1],
                in1=xpl[p % NABUF][:, c_ * cn:(c_ + 1) * cn],
                op0=ALU.mult, op1=ALU.add)
            nc.gpsimd.dma_start(out_d[:, 2 * p, c_ * cw:(c_ + 1) * cw, :],
                                ot[pb][64:128, c_ * cn:(c_ + 1) * cn])
            nc.gpsimd.dma_start(out_d[:, 2 * p + 1, c_ * cw:(c_ + 1) * cw, :],
                                ot[pb][0:64, c_ * cn:(c_ + 1) * cn])

    for p in range(NPAIR):
        pb = p % 2
        ae_, ao_ = ae[p % NABUF], ao[p % NABUF]
        qs_, ot_ = qs[pb], ot[pb]

        for j in range(NBLK):
            kqE = psum.tile([128, BN], F32, tag="kqE", name="kqE")
            kqO = psum.tile([128, BN], F32, tag="kqO", name="kqO")
            V = psumv.tile([128, BN], F32, tag="V", name="V")

            def mov(tl, dy, dx, lo=None):
                r0 = j * RB + dy
                if lo is None:
                    return tl[:, r0:r0 + RB, dx:dx + H]
                if lo:
                    return tl[0:64, r0:r0 + RB, dx:dx + H]
                return tl[64:128, r0:r0 + RB, dx:dx + H]

            # kq pair taps: K=128, full array, serial
            for dy in range(3):
                nc.tensor.matmul(kqE[:, :], wkqp_t[:, 0, dy, :],
                                 mov(ae_, dy, 0), start=(dy == 0), stop=False)
                nc.tensor.matmul(kqO[:, :], wkqp_t[:, 1, dy, :],
                                 mov(ao_, dy, 0), start=(dy == 0), stop=False)
            # kq single taps: K=64, even rows 0-63 / odd rows 64-127,
            # 2-way row-tiled concurrent
            for dy in range(3):
                nc.tensor.matmul(kqE[:, :], wkqs_t[0:64, dy, :],
                                 mov(ae_, dy, 1, lo=True),
                                 start=False, stop=(dy == 2))
                nc.tensor.matmul(kqO[:, :], wkqs_t[64:128, dy, :],
                                 mov(ao_, dy, 1, lo=False),
                                 start=False, stop=(dy == 2))
            # v pair taps: K=128 M=64, even cols 0-63 / odd cols 64-127,
            # 2-way col-tiled concurrent
            for dy in range(3):
                nc.tensor.matmul(V[0:64, :], wvp_t[:, 0, dy, :],
                                 mov(ae_, dy, 0), start=(dy == 0), stop=False,
                                 skip_group_check=True)
                nc.tensor.matmul(V[64:128, :], wvp_t[:, 1, dy, :],
                                 mov(ao_, dy, 0), start=(dy == 0), stop=False,
                                 skip_group_check=True)
            # v single taps: K=64 M=64, quadrants (0,0) and (64,64)
            for dy in range(3):
                nc.tensor.matmul(V[0:64, :], wvs_t[0:64, dy, :],
                                 mov(ae_, dy, 1, lo=True),
                                 start=False, stop=(dy == 2),
                                 skip_group_check=True)
                nc.tensor.matmul(V[64:128, :], wvs_t[64:128, dy, :],
                                 mov(ao_, dy, 1, lo=False),
                                 start=False, stop=(dy == 2),
                                 skip_group_check=True)

            # PSUM evacuation on ScalarE with bias folding
            nc.scalar.activation(qs_[64:128, j * BN:(j + 1) * BN],
                                 kqE[64:128, :], ACT.Identity,
                                 bias=bias_t[64:128, 0:1])
            nc.scalar.activation(qs_[0:64, j * BN:(j + 1) * BN],
                                 kqO[0:64, :], ACT.Identity,
                                 bias=bias_t[0:64, 0:1])
            vsb = vpool.tile([128, BN], F32, tag="vsb", name="vsb")
            nc.scalar.activation(vsb[:, :], V[:, :], ACT.Identity,
                                 bias=bias_t[:, 1:2],
                                 accum_out=vsum[pb][:, j:j + 1])

            # fused (gamma*k)*v' multiply + pixel-sum on DVE
            nc.vector.scalar_tensor_tensor(
                out=scr[0:64, :], in0=kqE[0:64, :], scalar=bias_t[0:64, 3:4],
                in1=vsb[0:64, :], op0=ALU.mult, op1=ALU.mult,
                accum_out=sparts[pb][0:64, j:j + 1])
            if kvo_mode == "stt":
                nc.vector.scalar_tensor_tensor(
                    out=scr[64:128, :], in0=kqO[64:128, :],
                    scalar=bias_t[64:128, 3:4],
                    in1=vsb[64:128, :], op0=ALU.mult, op1=ALU.mult,
                    accum_out=sparts[pb][64:128, j:j + 1])
            else:
                nc.vector.scalar_tensor_tensor(
                    out=scr[64:128, :], in0=kqO[64:128, :],
                    scalar=bias_t[64:128, 3:4],
                    in1=vsb[64:128, :], op0=ALU.mult, op1=ALU.mult)
                nc.vector.reduce_sum(sparts[pb][64:128, j:j + 1],
                                     scr[64:128, :],
                                     axis=mybir.AxisListType.X)

            # previous pair's final pass, deferred so the DVE queue never
            # holds this pair's psum turnaround behind it; prefetch loads
            # likewise emitted mid-pair
            if j == 1 and p > 0:
                emit_final(p - 1)
            if j == 3 and p + 2 < NPAIR:
                load_pair(p + 2)

        # s = gamma*sum(k*v') + gamma*bk*sum(v'), then swap halves so the
        # even-slice s reaches partitions 64-127 (q/x-plain live there)
        nc.vector.reduce_sum(t1[pb][:, :], sparts[pb][:, :],
                             axis=mybir.AxisListType.X)
        nc.vector.reduce_sum(vs1[pb][:, :], vsum[pb][:, :],
                             axis=mybir.AxisListType.X)
        nc.vector.scalar_tensor_tensor(
            out=sg[pb][:, :], in0=vs1[pb][:, :], scalar=bias_t[:, 2:3],
            in1=t1[pb][:, :], op0=ALU.mult, op1=ALU.add)
        nc.scalar.dma_start(sf[pb][64:128, :], sg[pb][0:64, :])
        nc.scalar.dma_start(sf[pb][0:64, :], sg[pb][64:128, :])

    emit_final(NPAIR - 1)


_CACHE = {}


def _build():
    if "nc" in _CACHE:
        return _CACHE["nc"]
    nc = bacc.Bacc("TRN2", target_bir_lowering=False, debug=False,
                   enable_asserts=False, num_devices=8)
    x_d = nc.dram_tensor("x", (C, T, W, H), BF16, kind="ExternalInput").ap()
    xp_d = nc.dram_tensor("xp", (T, 128, WP, HP), BF16,
                          kind="ExternalInput").ap()
    wkqp_d = nc.dram_tensor("wkqp", (128, 2, 3, 128), BF16,
                            kind="ExternalInput").ap()
    wkqs_d = nc.dram_tensor("wkqs", (128, 3, 128), BF16,
                            kind="ExternalInput").ap()
    wvp_d = nc.dram_tensor("wvp", (128, 2, 3, 64), BF16,
                           kind="ExternalInput").ap()
    wvs_d = nc.dram_tensor("wvs", (128, 3, 64), BF16,
                           kind="ExternalInput").ap()
    bias_d = nc.dram_tensor("bias", (128, 4), F32, kind="ExternalInput").ap()
    out_d = nc.dram_tensor("out", (C, T, W, H), BF16,
                           kind="ExternalOutput").ap()
    from contextlib import ExitStack
    with tile.TileContext(nc) as tc, ExitStack() as ctx:
        _emit(nc, tc, x_d, xp_d, wkqp_d, wkqs_d, wvp_d, wvs_d, bias_d, out_d,
              ctx)
    nc.compile()
    _CACHE["nc"] = nc
    return nc


def run_spmd(x, wq, wk, wv, bq, bk, bv, gamma, trace=False, **kw):
    nc = _build()
    wkqp, wkqs, wvp, wvs = _pack_weights(
        np.asarray(wq, np.float32), np.asarray(wk, np.float32),
        np.asarray(wv, np.float32))
    # stationary tiles are [128(K), ...free...]: transpose packed
    # [..., K, M] so K is the partition dim
    wkqp = np.ascontiguousarray(wkqp.transpose(2, 0, 1, 3))   # [128,2,3,128]
    wkqs = np.ascontiguousarray(wkqs.transpose(1, 0, 2))      # [128,3,128]
    wvp = np.ascontiguousarray(wvp.transpose(2, 0, 1, 3))     # [128,2,3,64]
    wvs = np.ascontiguousarray(wvs.transpose(1, 0, 2))        # [128,3,64]

    gam = np.float32(np.asarray(gamma).reshape(-1)[0])
    bias = np.zeros((128, 4), np.float32)
    bias[0:64, 0] = bias[64:128, 0] = np.asarray(bq, np.float32)
    bias[0:64, 1] = bias[64:128, 1] = np.asarray(bv, np.float32)
    bias[0:64, 2] = bias[64:128, 2] = np.asarray(bk, np.float32) * gam
    bias[:, 3] = gam

    xb = _bf16(x)
    # host pre-padded per-slice layout [T, 128, WP, HP]: even slices
    # [shift | plain], odd slices [plain | shift] on the partition halves
    zpad = np.zeros((B, T, C, WP, HP), ml_dtypes.bfloat16)
    zsh = np.zeros((B, T, C, WP, HP), ml_dtypes.bfloat16)
    xt = xb.transpose(0, 2, 1, 3, 4)            # [B, T, C, W, H]
    zpad[:, :, :, 1:1 + W, 1:1 + H] = xt
    zsh[:, :, :, 1:1 + W, 0:H] = xt
    xp = np.empty((B, T, 128, WP, HP), ml_dtypes.bfloat16)
    xp[:, 0::2, 0:64] = zsh[:, 0::2]
    xp[:, 0::2, 64:128] = zpad[:, 0::2]
    xp[:, 1::2, 0:64] = zpad[:, 1::2]
    xp[:, 1::2, 64:128] = zsh[:, 1::2]
    in_maps = [
        {"x": np.ascontiguousarray(xb[b]), "xp": np.ascontiguousarray(xp[b]),
         "wkqp": wkqp, "wkqs": wkqs,
         "wvp": wvp, "wvs": wvs, "bias": bias}
        for b in range(B)
    ]
    res = bass_utils.run_bass_kernel_spmd(
        nc, in_maps, core_ids=list(range(B)), trace=trace, **kw)
    out = np.stack([np.asarray(res.results[b]["out"]).astype(np.float32)
                    for b in range(B)], axis=0)
    return out, res


def kernel(x, wq, wk, wv, bq, bk, bv, gamma):
    out, _ = run_spmd(x, wq, wk, wv, bq, bk, bv, gamma)
    return out
